# revision 1
# baseline (speedup 1.0000x reference)
"""CyclicalAttention Trainium2 kernel — 8-core SPMD, head-sharded.

Sharding: 16 heads / 8 cores = 2 heads per core (both batches on every
core).  Per core (Megatron-style):
  - column-parallel Q/K/V projections for its 128-dim head slice
  - full attention for its 2 heads x 2 batches
  - row-parallel slice of the output projection -> partial y
Host sums the 8 partial outputs and adds bo.

Kernel tricks:
  - the rank-1 cyclical bias is folded into the score matmul by
    augmenting the head dim 64 -> 65 (Q_aug row 64 = scale*u,
    K_aug row 64 = u, u = seq-normalized cycle embedding)
  - the softmax denominator is folded into the PV matmul by augmenting
    V with a ones column (row 64 of the PV output is the denominator)
  - all activations stay d-major ("transposed") so the only transpose
    needed is V (32 PE transposes per core)
"""

import math

import numpy as np
import ml_dtypes

D_MODEL = 1024
N_HEADS = 16
HEAD_DIM = 64
B, S = 2, 2048
EPS = 1e-12
N_CORES = 8
HPC = N_HEADS // N_CORES          # heads per core = 2
DC = HPC * HEAD_DIM               # per-core model-dim slice = 128
NSEQ = B * S                      # 4096
P = 128
BF16 = ml_dtypes.bfloat16

_CACHE = {}
YT_BF16 = True  # bf16 partial outputs: halves the y DMA traffic


def _build_module(repeat=1, probe=None):
    import contextlib

    import concourse.bacc as bacc
    import concourse.mybir as mybir
    import concourse.tile as tile
    from concourse import library_config
    from concourse.masks import make_identity

    f32 = mybir.dt.float32
    bf16 = mybir.dt.bfloat16
    Exp = mybir.ActivationFunctionType.Exp
    mult = mybir.AluOpType.mult
    add = mybir.AluOpType.add

    nc = bacc.Bacc(
        "TRN2",
        target_bir_lowering=False,
        debug=False,
        enable_asserts=False,
        num_devices=N_CORES,
    )

    if probe == "cal":
        # loop-overhead calibration: same I/O surface, near-empty body
        for nm, shp, dt_ in (
            ("xt", [D_MODEL, NSEQ], mybir.dt.bfloat16),
            ("wq_t", [D_MODEL, DC], mybir.dt.bfloat16),
            ("wk_t", [D_MODEL, DC], mybir.dt.bfloat16),
            ("wv_t", [D_MODEL, DC], mybir.dt.bfloat16),
            ("wo_t", [DC, D_MODEL], mybir.dt.bfloat16),
            ("qaug", [HPC, NSEQ], mybir.dt.bfloat16),
            ("kaug", [HPC, NSEQ], mybir.dt.bfloat16),
            ("bq8", [DC, 1], mybir.dt.float32),
            ("bk", [DC, 1], mybir.dt.float32),
        ):
            nc.dram_tensor(nm, shp, dt_, kind="ExternalInput")
        out_d = nc.dram_tensor(
            "probe", [P, 64], mybir.dt.bfloat16, kind="ExternalOutput"
        ).ap()
        with tile.TileContext(nc) as tc:
            with (
                tc.tile_pool(name="p", bufs=2) as pool,
                tc.For_i(0, repeat, 1) if repeat > 1 else contextlib.nullcontext(),
            ):
                t = pool.tile([P, 64], mybir.dt.bfloat16)
                nc.vector.memset(t[:], 1.0)
                nc.sync.dma_start(out_d, t[:])
        nc.compile()
        return nc

    xt_d = nc.dram_tensor("xt", [D_MODEL, NSEQ], bf16, kind="ExternalInput").ap()
    wq_d = nc.dram_tensor("wq_t", [D_MODEL, DC], bf16, kind="ExternalInput").ap()
    wk_d = nc.dram_tensor("wk_t", [D_MODEL, DC], bf16, kind="ExternalInput").ap()
    wv_d = nc.dram_tensor("wv_t", [D_MODEL, DC], bf16, kind="ExternalInput").ap()
    wo_d = nc.dram_tensor("wo_t", [DC, D_MODEL], bf16, kind="ExternalInput").ap()
    qaug_d = nc.dram_tensor("qaug", [HPC, NSEQ], bf16, kind="ExternalInput").ap()
    kaug_d = nc.dram_tensor("kaug", [HPC, NSEQ], bf16, kind="ExternalInput").ap()
    bq8_d = nc.dram_tensor("bq8", [DC, 1], f32, kind="ExternalInput").ap()
    bk_d = nc.dram_tensor("bk", [DC, 1], f32, kind="ExternalInput").ap()
    yt_dt = bf16 if YT_BF16 else f32
    yt_d = nc.dram_tensor("yt", [D_MODEL, NSEQ], yt_dt, kind="ExternalOutput").ap()
    probe_d = (
        nc.dram_tensor("probe", [P, 6 * NSEQ], bf16, kind="ExternalOutput").ap()
        if probe
        else None
    )

    KT = D_MODEL // P   # 8 contraction tiles for the projections
    NCH = NSEQ // 512   # 8 seq chunks of 512
    SCT = S // P        # 16 k-tiles per (b, h) in attention

    with tile.TileContext(nc) as tc:
        with (
            tc.tile_pool(name="consts", bufs=1) as consts,
            tc.tile_pool(name="xtp", bufs=1) as xtp,
            tc.tile_pool(name="wp", bufs=1) as wp,
            tc.tile_pool(name="acts", bufs=1) as acts,
            tc.tile_pool(name="ep", bufs=6) as ep,
            tc.tile_pool(name="rp", bufs=2) as rp,
            tc.tile_pool(name="yp", bufs=6) as yp,
            tc.tile_pool(name="ps_sc", bufs=3, space="PSUM") as ps_sc,
            tc.tile_pool(name="ps_pv", bufs=1, space="PSUM") as ps_pv,
            tc.For_i(0, repeat, 1) if repeat > 1 else contextlib.nullcontext(),
        ):
            # ---- constants / weights / biases ----
            # weight/bias/aug DMAs ride the Activation HWDGE queue (idle at
            # kernel start) so they don't serialize behind the big x^T load
            nc.gpsimd.load_library(library_config.attn)

            wq_sb = consts.tile([P, KT, DC], bf16)
            wk_sb = consts.tile([P, KT, DC], bf16)
            wv_sb = consts.tile([P, KT, DC], bf16)
            nc.scalar.dma_start(wq_sb[:], wq_d.rearrange("(t p) m -> p t m", p=P))
            nc.scalar.dma_start(wk_sb[:], wk_d.rearrange("(t p) m -> p t m", p=P))
            nc.scalar.dma_start(wv_sb[:], wv_d.rearrange("(t p) m -> p t m", p=P))
            wo_sb = consts.tile([DC, D_MODEL], bf16)
            nc.scalar.dma_start(wo_sb[:], wo_d)
            bq8_sb = consts.tile([DC, 1], f32)
            bk_sb = consts.tile([DC, 1], f32)
            nc.scalar.dma_start(bq8_sb[:], bq8_d)
            nc.scalar.dma_start(bk_sb[:], bk_d)

            # x^T, 8 tiles of [128, 4096], split across both HWDGE queues
            xt_sb = [xtp.tile([P, NSEQ], bf16, tag=f"xt{t}", name=f"xt{t}") for t in range(KT)]
            for t in range(KT):
                eng = nc.sync if t % 2 == 0 else nc.scalar
                eng.dma_start(
                    xt_sb[t][:], xt_d.rearrange("(t p) n -> t p n", p=P)[t]
                )

            # ---- activations (persistent SBUF) ----
            # Q^T / K^T augmented per local head: [65, 4096]
            qt_sb = [acts.tile([HEAD_DIM + 1, NSEQ], bf16, tag=f"qt{h}", name=f"qt{h}") for h in range(HPC)]
            kt_sb = [acts.tile([HEAD_DIM + 1, NSEQ], bf16, tag=f"kt{h}", name=f"kt{h}") for h in range(HPC)]
            for h in range(HPC):
                nc.scalar.dma_start(qt_sb[h][HEAD_DIM : HEAD_DIM + 1, :], qaug_d[h : h + 1, :])
                nc.scalar.dma_start(kt_sb[h][HEAD_DIM : HEAD_DIM + 1, :], kaug_d[h : h + 1, :])
            # V_aug: [128(k), bh, kt, 65]; col 64 = ones (denominator trick)
            v_all = acts.tile([P, B * HPC, SCT, HEAD_DIM + 1], bf16, tag="vall")
            nc.vector.memset(v_all[:, :, :, HEAD_DIM : HEAD_DIM + 1], 1.0)
            # attention output (d-major), split per batch for o-proj overlap
            ao_sb = [acts.tile([DC, S], bf16, tag=f"ao{b}", name=f"ao{b}") for b in range(B)]

            # ---- phase 1: projections (chunk emitters) ----
            def proj_chunk(w_sb, post, n):
                ps = ps_sc.tile([P, 1024], f32, tag="mm", name="ps_p")
                pss = ps[:, :512]
                for t in range(KT):
                    nc.tensor.matmul(
                        pss,
                        w_sb[:, t, :],
                        xt_sb[t][:, n * 512 : (n + 1) * 512],
                        start=(t == 0),
                        stop=(t == KT - 1),
                    )
                post(n, pss)

            def q_post(n, pss):
                for h in range(HPC):
                    nc.vector.tensor_scalar(
                        qt_sb[h][:HEAD_DIM, n * 512 : (n + 1) * 512],
                        pss[h * HEAD_DIM : (h + 1) * HEAD_DIM, :],
                        0.125,
                        bq8_sb[h * HEAD_DIM : (h + 1) * HEAD_DIM, :],
                        mult,
                        add,
                    )

            def k_post(n, pss):
                for h in range(HPC):
                    nc.vector.tensor_scalar_add(
                        kt_sb[h][:HEAD_DIM, n * 512 : (n + 1) * 512],
                        pss[h * HEAD_DIM : (h + 1) * HEAD_DIM, :],
                        bk_sb[h * HEAD_DIM : (h + 1) * HEAD_DIM, :],
                    )

            # Q and K interleaved so the first attention tiles unblock early
            # (deps are region-granular); V is deferred into the drain queue.
            if probe != "load":
                for n in range(NCH):
                    proj_chunk(wq_sb, q_post, n)
                    proj_chunk(wk_sb, k_post, n)

            # ---- phase 2 emitters: V projection directly in [k, dv] layout
            # (stationary = x^T chunk, moving = Wv^T; no transposes needed;
            # bv is folded out on the host since attn rows sum to 1) ----
            def vnat_chunk(sc):
                def emit():
                    b, kt = divmod(sc, SCT)
                    ps = ps_sc.tile([P, 1024], f32, tag="mm", name="ps_v")
                    pss = ps[:, :DC]
                    for t in range(KT):
                        nc.tensor.matmul(
                            pss,
                            xt_sb[t][:, sc * P : (sc + 1) * P],
                            wv_sb[:, t, :],
                            start=(t == 0),
                            stop=(t == KT - 1),
                        )
                    for h in range(HPC):
                        nc.vector.tensor_copy(
                            v_all[:, b * HPC + h, kt, :HEAD_DIM],
                            pss[:, h * HEAD_DIM : (h + 1) * HEAD_DIM],
                        )

                return emit

            # ---- output projection chunk emitters (drained into the
            # attention loops so they overlap on otherwise-idle slack) ----
            def oproj_chunk(b, ec, sc2):
                def emit():
                    ps = ps_sc.tile([P, 1024], f32, tag="mm", name="ps_o")
                    pss = ps[:, :512]
                    nc.tensor.matmul(
                        pss,
                        wo_sb[:, ec * P : (ec + 1) * P],
                        ao_sb[b][:, sc2 * 512 : (sc2 + 1) * 512],
                        start=True,
                        stop=True,
                    )
                    y_sb = yp.tile([P, 512], yt_dt, tag="y", name="y_sb")
                    # last batch's chunks run in the tail when the scalar
                    # engine is done with exp: spread copy + DMA onto it
                    if b == B - 1 and ec % 2 == 1:
                        nc.scalar.copy(y_sb[:], pss)
                        dma_eng = nc.scalar
                    else:
                        nc.vector.tensor_copy(y_sb[:], pss)
                        dma_eng = nc.sync
                    dma_eng.dma_start(
                        yt_d[
                            ec * P : (ec + 1) * P,
                            b * S + sc2 * 512 : b * S + (sc2 + 1) * 512,
                        ],
                        y_sb[:],
                    )

                return emit

            pending = []

            def drain(n=1):
                for _ in range(min(n, len(pending))):
                    pending.pop(0)()

            # V work drains into the front of the attention phase, kept just
            # ahead of the PV matmuls that consume it (V(kt) feeds PV(.., kt)).
            for sc in range(B * SCT):
                pending.append(vnat_chunk(sc))

            # ---- phase 3: attention per (b, h, q-half) ----
            def attn_unit(b, h, qh):
                col0 = b * S
                pv = ps_pv.tile([HEAD_DIM + 1, 1024], f32, tag="pv", name="pv")
                for kt in range(SCT):
                    drain(2 if kt < 8 else 1)
                    ps = ps_sc.tile([P, 1024], f32, tag="mm", name="ps_s")
                    for c in range(2):
                        q0 = col0 + qh * 1024 + c * 512
                        nc.tensor.matmul(
                            ps[:, c * 512 : (c + 1) * 512],
                            kt_sb[h][:, col0 + kt * P : col0 + (kt + 1) * P],
                            qt_sb[h][:, q0 : q0 + 512],
                            start=True,
                            stop=True,
                        )
                    e = ep.tile([P, 1024], bf16, tag="e", name="e")
                    nc.scalar.activation(e[:], ps[:], Exp)
                    for c in range(2):
                        nc.tensor.matmul(
                            pv[:, c * 512 : (c + 1) * 512],
                            v_all[:, b * HPC + h, kt, :],
                            e[:, c * 512 : (c + 1) * 512],
                            start=(kt == 0),
                            stop=(kt == SCT - 1),
                        )
                # normalize: out = pv[0:64] / pv[64]
                r_sb = rp.tile([1, 1024], f32, tag="r", name="r_sb")
                nc.vector.reciprocal(r_sb[:], pv[HEAD_DIM : HEAD_DIM + 1, :])
                rb = rp.tile([HEAD_DIM, 1024], f32, tag="rb", name="rb")
                nc.gpsimd.partition_broadcast(rb[:], r_sb[:])
                nc.vector.tensor_tensor(
                    ao_sb[b][
                        h * HEAD_DIM : (h + 1) * HEAD_DIM,
                        qh * 1024 : (qh + 1) * 1024,
                    ],
                    pv[:HEAD_DIM, :],
                    rb[:],
                    mult,
                )
            if probe in ("load", "qk"):
                pending.clear()
            if probe not in ("proj", "load", "qk"):
                for b in range(B):
                    for h in range(HPC):
                        for qh in range(2):
                            attn_unit(b, h, qh)
                    if probe is None:
                        # batch b's heads complete -> queue its o-proj chunks
                        pending.extend(
                            oproj_chunk(b, ec, sc2)
                            for ec in range(D_MODEL // P)
                            for sc2 in range(S // 512)
                        )
            drain(len(pending))

            if probe == "load":
                t0 = ep.tile([P, 1024], bf16, tag="e", name="t0")
                nc.vector.tensor_copy(t0[:], xt_sb[7][:, :1024])
                nc.sync.dma_start(probe_d[:, :1024], t0[:])
            elif probe == "qk":
                for h in range(HPC):
                    nc.sync.dma_start(probe_d[:65, 0:NSEQ], qt_sb[h][:])
                    nc.sync.dma_start(probe_d[:65, NSEQ : 2 * NSEQ], kt_sb[h][:])
            elif probe == "proj":
                for h in range(HPC):
                    nc.sync.dma_start(probe_d[:65, 0:NSEQ], qt_sb[h][:])
                    nc.sync.dma_start(probe_d[:65, NSEQ : 2 * NSEQ], kt_sb[h][:])
                nc.sync.dma_start(
                    probe_d[:, 2 * NSEQ : 2 * NSEQ + B * HPC * SCT * 65],
                    v_all[:].rearrange("p a b c -> p (a b c)"),
                )
            elif probe == "attn":
                for b in range(B):
                    nc.sync.dma_start(
                        probe_d[:, b * S : (b + 1) * S], ao_sb[b][:]
                    )

    nc.compile()
    return nc


def _get_module(repeat=1, probe=None):
    key = f"nc{repeat}{probe or ''}"
    if key not in _CACHE:
        _CACHE[key] = _build_module(repeat, probe)
    return _CACHE[key]


def _host_prep(x, temporal_features, wq, bq, wk, bk, wv, bv, wo, bo, wc, bc, cycle_scale):
    """Shard/lay out the inputs for the 8 cores."""
    x = np.asarray(x, np.float32)
    xt = np.ascontiguousarray(x.reshape(NSEQ, D_MODEL).T).astype(BF16)

    # cycle embedding (tiny): [B, S, H] -> [B, H, S], seq-normalized
    ce = (
        np.asarray(temporal_features, np.float32).reshape(NSEQ, -1) @ np.asarray(wc, np.float32).T
        + np.asarray(bc, np.float32)
    ).reshape(B, S, N_HEADS).transpose(0, 2, 1)
    nrm = np.maximum(np.linalg.norm(ce, axis=-1, keepdims=True), EPS)
    cn = ce / nrm  # [B, H, S]
    cs = np.asarray(cycle_scale, np.float32)

    in_maps = []
    for c in range(N_CORES):
        rows = slice(c * DC, (c + 1) * DC)
        qaug = np.empty((HPC, NSEQ), np.float32)
        kaug = np.empty((HPC, NSEQ), np.float32)
        for h in range(HPC):
            gh = c * HPC + h
            for b in range(B):
                qaug[h, b * S : (b + 1) * S] = cs[gh] * cn[b, gh]
                kaug[h, b * S : (b + 1) * S] = cn[b, gh]
        in_maps.append(
            {
                "xt": xt,
                "wq_t": np.ascontiguousarray(np.asarray(wq, np.float32)[rows].T).astype(BF16),
                "wk_t": np.ascontiguousarray(np.asarray(wk, np.float32)[rows].T).astype(BF16),
                "wv_t": np.ascontiguousarray(np.asarray(wv, np.float32)[rows].T).astype(BF16),
                "wo_t": np.ascontiguousarray(np.asarray(wo, np.float32)[:, rows].T).astype(BF16),
                "qaug": qaug.astype(BF16),
                "kaug": kaug.astype(BF16),
                "bq8": (np.asarray(bq, np.float32)[rows] * 0.125).reshape(DC, 1).copy(),
                "bk": np.asarray(bk, np.float32)[rows].reshape(DC, 1).copy(),
            }
        )
    return in_maps


def kernel(**inputs):
    from concourse import bass_utils

    nc = _get_module()
    in_maps = _host_prep(**inputs)
    res = bass_utils.run_bass_kernel_spmd(nc, in_maps, core_ids=list(range(N_CORES)))
    yt = np.zeros((D_MODEL, NSEQ), np.float64)
    for r in res.results:
        yt += r["yt"].astype(np.float64)
    # bv is folded out of the device kernel: attn rows sum to 1, so
    # attn@(V+bv) @ wo.T = attn@V @ wo.T + bv @ wo.T
    bias = np.asarray(inputs["bo"], np.float64) + np.asarray(
        inputs["bv"], np.float64
    ) @ np.asarray(inputs["wo"], np.float64).T
    y = yt.T.reshape(B, S, D_MODEL) + bias
    return y.astype(np.float32)

